# revision 61
# baseline (speedup 1.0000x reference)
"""Multi-head attention TRN2 kernel (B=4, S=2048, D=1024, H=16).

Sharding: 8 cores = (batch, head-half) pairs. Core c handles batch c//2
and heads (c%2)*8..(c%2)*8+8 for ALL 2048 queries. Each core computes a
partial output (its 8 heads' contribution through the output projection);
the host sums the two partials per batch (the O-projection is linear in
the head dimension), adding bo exactly once (only the even core gets a
nonzero bo input).

Mask compression: the mask is per-key 0/1 with ~half the keys masked to
-1e9 (=> exp underflows to exactly 0, contributing nothing to softmax
numerator or denominator). The host drops masked keys, compacting K/V to
the kept columns, padded per-batch to a common multiple of 128. Pad
columns carry a -1e9 bias so their exp is 0 too. This roughly halves all
attention-side work (scores, exp, AV) and the K/V projections.

Per-core dataflow (contraction dim always on SBUF partitions; PE computes
C[M,N] = lhsT[K,M].T @ rhs[K,N]; everything the PE consumes is bf16):

  phase A:  KT[dout, k]  = wk.T-chunks x XkT   (dout = 512 local dims)
            V[k, dh]     = XvT-chunks x wv     head-strided [k, 8*(DH+1)]
                           with a ones column per head (denominator rows).
  per q-block qb (512 q rows, 4 blocks):
    A2:     QT[dout, q]  = wq'.T-chunks x XqT  (wq' = wq/sqrt(DH), host)
    B:      for each local head pair pr (4 pairs, row-packed 0-63/64-127):
              for each k-chunk kc:
                scoresT[k,q] = KT_h-slice.T x QT_h   (contraction dh=64)
                PT = exp(scoresT + m[kc])            (ACT bias = mask col)
                po[hp][dh+1, q] += (V_h | 1).T x PT  (accum over kc, PSUM)
              row dh of po = softmax denominators; normalize via
              reciprocal_approx_fast (DVE) + partition_broadcast (GPSIMD)
              + one DVE mul per head -> OT bf16
    C:      out[q, n] accumulated in PSUM over the 4 pairs
            (start/stop matmul accumulation, no DVE adds), then one
            copy per chunk (alternating DVE/ACT) -> SBUF -> DRAM.
"""

import numpy as np
import ml_dtypes

import concourse.bacc as bacc
import concourse.mybir as mybir
import concourse.tile as tile
from concourse.bass_utils import run_bass_kernel_spmd

F32 = mybir.dt.float32
BF16 = mybir.dt.bfloat16

B, S, D, H = 4, 2048, 1024, 16
DH = D // H
P = 128
NCORES = 8
HLOC = H // 2          # heads per core
HD = HLOC * DH         # local head dims = 512
NEG = -1.0e9


def build_nc(skp, d=D, s_q=S, qblk=512, with_bias=False, finalize=True):
    """Per-core Bass program. skp = padded kept-key count (mult of 128)."""
    dh = DH
    ndi = d // P           # contraction chunks over model dim (8)
    ndc = HD // P          # local out-dim chunks (4) == head pairs
    nkc = skp // P         # key chunks
    nqb = s_q // qblk      # q blocks (4)
    npr = HLOC // 2        # local head pairs (4)
    Exp = mybir.ActivationFunctionType.Exp

    nc = bacc.Bacc()
    xqt_d = nc.dram_tensor("xqt", [d, s_q], BF16, kind="ExternalInput")
    xkt_d = nc.dram_tensor("xkt", [d, skp], BF16, kind="ExternalInput")
    xvt_d = nc.dram_tensor("xvt", [d, skp], BF16, kind="ExternalInput")
    wq_d = nc.dram_tensor("wq", [d, HD], BF16, kind="ExternalInput")
    wk_d = nc.dram_tensor("wk", [d, HD], BF16, kind="ExternalInput")
    wv_d = nc.dram_tensor("wv", [d, HD], BF16, kind="ExternalInput")
    wo_d = nc.dram_tensor("wo", [HD, d], BF16, kind="ExternalInput")
    m_d = nc.dram_tensor("mrow", [P, nkc], F32, kind="ExternalInput")
    vones_d = nc.dram_tensor("vones", [P, HLOC], BF16, kind="ExternalInput")
    if with_bias:
        ones_d = nc.dram_tensor("ones", [1, qblk], BF16, kind="ExternalInput")
        bias_d = nc.dram_tensor("biases", [1, 3 * HD + d], BF16,
                                kind="ExternalInput")
    out_d = nc.dram_tensor("out", [s_q, d], F32, kind="ExternalOutput")

    mm = nc.tensor.matmul

    def kslabs():
        o = 0
        while o < skp:
            w = min(256, skp - o)
            yield o, w
            o += w

    with tile.TileContext(nc) as tc:
        with (
            tc.tile_pool(name="persist", bufs=1) as pp,
            tc.tile_pool(name="small", bufs=1) as sp,
        ):
            m_sb = sp.tile([P, nkc], F32, tag="m")
            nc.sync.dma_start(m_sb[:, :], m_d[:, :])
            if with_bias:
                ones_sb = sp.tile([1, qblk], BF16, tag="ones")
                bias_sb = sp.tile([1, 3 * HD + d], BF16, tag="bias")
                nc.sync.dma_start(ones_sb[:, :], ones_d[:, :])
                nc.sync.dma_start(bias_sb[:, :], bias_d[:, :])

            kt_t = [pp.tile([P, skp], BF16, tag=f"kt{i}", name=f"kt{i}")
                    for i in range(ndc)]
            v_t = [pp.tile([P, HLOC * (dh + 1)], BF16, tag=f"v{i}",
                           name=f"v{i}") for i in range(nkc)]
            wq_sb = [pp.tile([P, HD], BF16, tag=f"wq{i}", name=f"wq{i}")
                     for i in range(ndi)]
            wo_sb = [pp.tile([P, d], BF16, tag=f"wo{i}", name=f"wo{i}")
                     for i in range(ndc)]
            for i in range(ndi):
                nc.sync.dma_start(wq_sb[i][:, :], wq_d[i * P:(i + 1) * P, :])
            for i in range(ndc):
                nc.sync.dma_start(wo_sb[i][:, :], wo_d[i * P:(i + 1) * P, :])

            # Long-lived SBUF pools for the q-block loop are opened before
            # phase A so the first q-block's X stream prefetches under it.
            with (
                tc.tile_pool(name="qtp", bufs=2) as qtp,
                tc.tile_pool(name="xqp", bufs=2) as xqp,
                tc.tile_pool(name="otp", bufs=2) as otp,
                tc.tile_pool(name="ptp", bufs=3) as ptp,
                tc.tile_pool(name="rcp", bufs=2) as rcp,
                tc.tile_pool(name="pbp", bufs=2) as pbp,
                tc.tile_pool(name="oap", bufs=2) as oap,
            ):
                def xq_fetch(iqb):
                    xq_sl = xqp.tile([P, ndi, qblk], BF16, tag="xq",
                                     name=f"xq{iqb}")
                    nc.sync.dma_start(
                        xq_sl[:, :, :],
                        xqt_d[:, :].rearrange("(c p) s -> p c s", p=P)[
                            :, :, iqb * qblk:(iqb + 1) * qblk],
                    )
                    return xq_sl

                xq_next = xq_fetch(0)

                # ------- phase A: K and V projections, interleaved -------
                with (
                    tc.tile_pool(name="wkp", bufs=1) as wkp,
                    tc.tile_pool(name="xsp", bufs=2) as xsp,
                    tc.tile_pool(name="psA", bufs=2, space="PSUM") as psA,
                ):
                    wk_sb = [wkp.tile([P, HD], BF16, tag=f"wk{i}",
                                      name=f"wk{i}") for i in range(ndi)]
                    wv_sb = [wkp.tile([P, HD], BF16, tag=f"wv{i}",
                                      name=f"wv{i}") for i in range(ndi)]
                    for i in range(ndi):
                        nc.sync.dma_start(wk_sb[i][:, :],
                                          wk_d[i * P:(i + 1) * P, :])
                        nc.sync.dma_start(wv_sb[i][:, :],
                                          wv_d[i * P:(i + 1) * P, :])
                    for ks, ksl in kslabs():
                        xk_sl = xsp.tile([P, ndi, 256], BF16, tag="xk")
                        nc.sync.dma_start(
                            xk_sl[:, :, 0:ksl],
                            xkt_d[:, :].rearrange("(c p) s -> p c s", p=P)[
                                :, :, ks:ks + ksl],
                        )
                        xv_sl = xsp.tile([P, ndi, 256], BF16, tag="xv")
                        nc.sync.dma_start(
                            xv_sl[:, :, 0:ksl],
                            xvt_d[:, :].rearrange("(c p) s -> p c s", p=P)[
                                :, :, ks:ks + ksl],
                        )
                        for dc in range(ndc):
                            ps = psA.tile([P, 256], F32, tag="psk")
                            for di in range(ndi):
                                mm(ps[:, 0:ksl],
                                   wk_sb[di][:, dc * P:(dc + 1) * P],
                                   xk_sl[:, di, 0:ksl],
                                   start=(di == 0),
                                   stop=(di == ndi - 1 and not with_bias))
                            if with_bias:
                                mm(ps[:, 0:ksl],
                                   bias_sb[0:1, HD + dc * P:HD + (dc + 1) * P],
                                   ones_sb[0:1, 0:ksl], start=False, stop=True)
                            nc.vector.tensor_copy(kt_t[dc][:, ks:ks + ksl],
                                                  ps[:, 0:ksl])
                        for kci in range(ksl // P):
                            kc = ks // P + kci
                            vt3 = v_t[kc].rearrange("p (g c) -> p g c",
                                                    c=dh + 1)
                            nc.sync.dma_start(vt3[:, :, dh:dh + 1],
                                              vones_d[:, :, None])
                            ps = psA.tile([P, HD], F32, tag="psv")
                            for di in range(ndi):
                                mm(ps[:, :],
                                   xv_sl[:, di, kci * P:(kci + 1) * P],
                                   wv_sb[di][:, :],
                                   start=(di == 0),
                                   stop=(di == ndi - 1 and not with_bias))
                            if with_bias:
                                mm(ps[:, :], ones_sb[0:1, 0:P],
                                   bias_sb[0:1, 2 * HD:3 * HD],
                                   start=False, stop=True)
                            nc.vector.tensor_copy(
                                vt3[:, :, 0:dh],
                                ps[:, :].rearrange("p (g c) -> p g c", c=dh),
                            )

                # ---------------- per q-block ----------------
                with (
                    tc.tile_pool(name="psx", bufs=2, space="PSUM") as psx,
                    tc.tile_pool(name="pss", bufs=2, space="PSUM") as pss,
                    tc.tile_pool(name="pso", bufs=2, space="PSUM") as pso,
                ):
                    def q_proj(iqb, xq_sl):
                        qt_t = [qtp.tile([P, qblk], BF16, tag=f"qt{i}",
                                         name=f"qt{iqb}_{i}")
                                for i in range(ndc)]
                        for dc in range(ndc):
                            ps = psx.tile([P, qblk], F32, tag="ps",
                                          name=f"psq{iqb}_{dc}")
                            for di in range(ndi):
                                mm(ps[:, :], wq_sb[di][:, dc * P:(dc + 1) * P],
                                   xq_sl[:, di, :],
                                   start=(di == 0),
                                   stop=(di == ndi - 1 and not with_bias))
                            if with_bias:
                                mm(ps[:, :], bias_sb[0:1, dc * P:(dc + 1) * P],
                                   ones_sb[0:1, 0:qblk],
                                   start=False, stop=True)
                            nc.vector.tensor_copy(qt_t[dc][:, :], ps[:, :])
                        return qt_t

                    def o_proj_steps(iqb, ot_loc, q0):
                        # Output projection of q-block iqb, one PE matmul
                        # (or drain) per yield, so the caller can lace it
                        # into the EXP-bound attention pipeline of the NEXT
                        # q-block where the PE has slack.
                        for qc in range(qblk // P):
                            oa = oap.tile([P, d], F32, tag="oa",
                                          name=f"oa{iqb}_{qc}")
                            for nh in range(d // 512):
                                ns = slice(nh * 512, (nh + 1) * 512)
                                ps = psx.tile([P, 512], F32, tag="ps",
                                              name=f"psO{iqb}_{qc}_{nh}")
                                for pr in range(npr):
                                    mm(ps[:, :],
                                       ot_loc[pr][:, qc * P:(qc + 1) * P],
                                       wo_sb[pr][:, ns],
                                       start=(pr == 0),
                                       stop=(pr == npr - 1 and not with_bias))
                                    yield
                                if with_bias:
                                    mm(ps[:, :], ones_sb[0:1, 0:P],
                                       bias_sb[0:1, 3 * HD + nh * 512:3 * HD + (nh + 1) * 512],
                                       start=False, stop=True)
                                if nh % 2 == 0:
                                    nc.vector.tensor_copy(oa[:, ns], ps[:, :])
                                else:
                                    nc.scalar.copy(oa[:, ns], ps[:, :])
                                yield
                            nc.sync.dma_start(
                                out_d[q0 + qc * P:q0 + (qc + 1) * P, :],
                                oa[:, :])

                    qt_t = q_proj(0, xq_next)
                    o_gen = None
                    for iqb in range(nqb):
                        q0 = iqb * qblk
                        if iqb + 1 < nqb:
                            xq_next = xq_fetch(iqb + 1)

                        # ---- B: attention ----
                        # PE order is software-pipelined one k-chunk deep:
                        # scores(kc+1) is emitted before AV(kc) so the PE is
                        # never head-of-line blocked on the EXP(kc) sem.
                        ot_t = [otp.tile([P, qblk], BF16, tag=f"ot{pr}",
                                         name=f"ot{iqb}_{pr}")
                                for pr in range(npr)]

                        def scores(pr, kc, qt_t=qt_t, iqb=iqb):
                            ss = pss.tile([P, 2 * qblk], F32, tag="ss",
                                          name=f"ss{iqb}_{pr}_{kc}")
                            for hp in range(2):
                                mm(ss[:, hp * qblk:(hp + 1) * qblk],
                                   kt_t[pr][hp * dh:(hp + 1) * dh,
                                            kc * P:(kc + 1) * P],
                                   qt_t[pr][hp * dh:(hp + 1) * dh, :],
                                   start=True, stop=True,
                                   tile_position=(hp * dh, 0))
                            return ss

                        for pr in range(npr):
                            po = [pso.tile([dh + 1, qblk], F32, tag="po",
                                           name=f"po{iqb}_{pr}_{j}")
                                  for j in range(2)]
                            if pr == 0:
                                ss_cur = scores(pr, 0)
                            for kc in range(nkc):
                                pt = ptp.tile([P, 2 * qblk], BF16, tag="pt",
                                              name=f"pt{iqb}_{pr}_{kc}")
                                nc.scalar.activation(pt[:, :], ss_cur[:, :],
                                                     Exp,
                                                     bias=m_sb[:, kc:kc + 1])
                                if kc + 1 < nkc:
                                    ss_cur = scores(pr, kc + 1)
                                elif pr + 1 < npr:
                                    ss_cur = scores(pr + 1, 0)
                                if o_gen is not None:
                                    next(o_gen, None)
                                for hp in range(2):
                                    hh = 2 * pr + hp
                                    mm(po[hp][:, :],
                                       v_t[kc][:, hh * (dh + 1):(hh + 1) * (dh + 1)],
                                       pt[:, hp * qblk:(hp + 1) * qblk],
                                       start=(kc == 0), stop=(kc == nkc - 1))
                            # Drain po to SBUF right away (frees the PSUM
                            # bank for the next pair), then normalize from
                            # SBUF. hp0's drain+extract rides the Scalar
                            # engine (idle at pair boundaries), hp1's the
                            # Vector engine.
                            for hp in range(2):
                                oraw = rcp.tile([dh + 1, qblk], F32,
                                                tag=f"oraw{hp}",
                                                name=f"oraw{iqb}_{pr}_{hp}")
                                dn = rcp.tile([1, qblk], F32, tag=f"dn{hp}",
                                              name=f"dn{iqb}_{pr}_{hp}")
                                nc.vector.tensor_copy(oraw[:, :],
                                                      po[hp][:, :])
                                nc.vector.tensor_copy(dn[:, :],
                                                      oraw[dh:dh + 1, :])
                                rc = rcp.tile([1, qblk], F32, tag=f"rc{hp}",
                                              name=f"rc{iqb}_{pr}_{hp}")
                                nc.vector.reciprocal_approx_fast(
                                    rc[:, :], dn[:, :])
                                pb = pbp.tile([dh, qblk], F32, tag=f"pb{hp}",
                                              name=f"pb{iqb}_{pr}_{hp}")
                                nc.gpsimd.partition_broadcast(
                                    pb[:, :], rc[:, :], channels=dh)
                                nc.vector.tensor_mul(
                                    ot_t[pr][hp * dh:(hp + 1) * dh, :],
                                    oraw[0:dh, :], pb[:, :])

                        # Drain any leftover O-projection work of the
                        # previous q-block, then Q projection for the NEXT
                        # q-block (fills the PE while the last pair's
                        # normalize chain completes), then hand this
                        # q-block's O projection to the next attention loop.
                        if o_gen is not None:
                            for _ in o_gen:
                                pass
                        if iqb + 1 < nqb:
                            qt_t = q_proj(iqb + 1, xq_next)
                        o_gen = o_proj_steps(iqb, ot_t, q0)

                    # ---- final q-block's output projection ----
                    for _ in o_gen:
                        pass
    if finalize:
        nc.finalize()
    return nc


def _bf16(a):
    return np.ascontiguousarray(np.asarray(a, np.float32).astype(ml_dtypes.bfloat16))


def make_in_maps(queries, keys, values, mask, wq, bq, wk, bk, wv, bv, wo, bo,
                 with_bias=False):
    """Host-side shard prep. Core c -> (batch c//2, head-half c%2)."""
    scale = 1.0 / np.sqrt(np.float32(DH))
    wq_s = np.asarray(wq, np.float32) * scale
    bq_s = np.asarray(bq, np.float32) * scale

    # mask compression: keep keys where mask == 0
    kept = [np.flatnonzero(np.asarray(mask[b, 0, 0, :]) == 0) for b in range(B)]
    for b in range(B):
        if kept[b].size == 0:          # degenerate: keep everything
            kept[b] = np.arange(S)
    skp = max(P, -(-max(k.size for k in kept) // P) * P)

    in_maps = []
    for c in range(NCORES):
        b, hh = divmod(c, 2)
        hs = slice(hh * HD, (hh + 1) * HD)
        idx = kept[b]
        nk = idx.size
        xk = np.zeros((D, skp), np.float32)
        xv = np.zeros((D, skp), np.float32)
        xk[:, :nk] = np.asarray(keys[b], np.float32).T[:, idx]
        xv[:, :nk] = np.asarray(values[b], np.float32).T[:, idx]
        mrow = np.full(skp, NEG, np.float32)
        mrow[:nk] = 0.0
        im = {
            "xqt": _bf16(np.asarray(queries[b], np.float32).T),
            "xkt": _bf16(xk),
            "xvt": _bf16(xv),
            "wq": _bf16(wq_s[:, hs]),
            "wk": _bf16(np.asarray(wk, np.float32)[:, hs]),
            "wv": _bf16(np.asarray(wv, np.float32)[:, hs]),
            "wo": _bf16(np.asarray(wo, np.float32)[hs, :]),
            "mrow": np.ascontiguousarray(mrow.reshape(-1, P).T),
            "vones": np.ones((P, HLOC), ml_dtypes.bfloat16),
        }
        if with_bias:
            bo_c = np.asarray(bo, np.float32) if hh == 0 else np.zeros(D, np.float32)
            im["ones"] = np.ones((1, 512), ml_dtypes.bfloat16)
            im["biases"] = _bf16(np.concatenate([
                bq_s[hs], np.asarray(bk, np.float32)[hs],
                np.asarray(bv, np.float32)[hs], bo_c]).reshape(1, -1))
        in_maps.append(im)
    return in_maps, skp


_CACHE = {}


def kernel(queries, keys, values, mask, wq, bq, wk, bk, wv, bv, wo, bo,
           _trace=False):
    with_bias = any(np.any(np.asarray(x)) for x in (bq, bk, bv, bo))
    in_maps, skp = make_in_maps(queries, keys, values, mask, wq, bq, wk, bk,
                                wv, bv, wo, bo, with_bias=with_bias)
    key = (skp, with_bias)
    if key not in _CACHE:
        _CACHE[key] = build_nc(skp, with_bias=with_bias)
    nc = _CACHE[key]
    res = run_bass_kernel_spmd(nc, in_maps, list(range(NCORES)), trace=_trace)
    out = np.empty((B, S, D), np.float32)
    for b in range(B):
        out[b] = res.results[2 * b]["out"] + res.results[2 * b + 1]["out"]
    if _trace:
        return out, res
    return out
